# revision 15
# baseline (speedup 1.0000x reference)
"""Trainium2 Bass kernel for nn_HFGA_54606214201918.

Computation (per batch element b, C=256 channels, L=4096 positions):
    xh  = (x[:, 0::2] - x[:, 1::2]) / sqrt(2)          # Haar high band  [C, L/2]
    q   = Wq @ x + bq                                  # [C, L]
    k   = Wk @ xh + bk                                 # [C, L/2]
    v   = Wv @ xh + bv                                 # [C, L/2]
    attn = softmax_over_keys((k^T q) / sqrt(C))        # [L/2, L]
    out = (v @ attn) * tanh(gate) + x

Sharding: data-parallel over batch B=8 across the 8 NeuronCores (one batch
element per core); weights are broadcast. No collectives needed.

Per-core algorithm highlights:
  - every matmul (projections, scores k^T q, yh = v @ E, softmax denominator
    Z = ones^T E) runs in fp8(e4m3) DoubleRow perf mode: the 256-wide
    contraction is one PE pass per chunk (2 fp8 weights per cell), halving
    tensor-engine time vs bf16. Measured end-to-end rel err ~4e-3.
  - scores are built in [keys m, queries l] layout; exp drains PSUM in
    2-bank pairs on the scalar engine with the 1/sqrt(C) scale and a
    constant -2 shift (keeps E inside fp8 range; cancels in the softmax)
    folded into the activation, writing fp8 E tiles [128,2,512] that
    directly serve as DoubleRow rhs planes.
  - softmax normalization applies to the SMALL output (v @ E): recip(Z) is
    split hi/lo into two bf16 rows and broadcast across partitions with two
    K=1 bf16 matmuls (fp32-accurate, ~6x faster than an fp32-mode matmul).
  - the kernel is fully software-pipelined: x streams in four 1 MB chunks
    on one HWDGE queue (sequential completion at full HBM rate) while
    projections AND the first attention l-tile chase the chunks; each
    l-tile's epilogue is emitted two score-groups into the next l-tile so
    the PE never waits on the normalization chain; dummy matmuls warm the
    PE HAM clock-gate during the initial DMA wait.
  - 1/sqrt(2) and tanh(gate) are folded into the weights on host;
    1/sqrt(C) and the -2 shift are folded into the exp activation.
"""
import sys

if '/opt/trn_rl_repo' not in sys.path:
    sys.path.insert(0, '/opt/trn_rl_repo')

import numpy as np

import concourse.bass as bass
import concourse.tile as tile
from concourse import bacc, mybir
from concourse import bass_utils

B, C, L = 8, 256, 4096
M = L // 2            # 2048 keys
P = 128               # partitions
CO = C // P           # 2 channel chunks
LB = 512              # l-tile (one PSUM bank of fp32)
NB = L // LB          # 8 l-tiles
MJ = M // P           # 16 key chunks of 128
G = MJ // 2           # 8 key groups of 256 (DoubleRow pairs)
XC = 4                # x DMA chunks of 1024 positions
XL = L // XC
INV_SQRT2 = 0.7071067811865476

F32 = mybir.dt.float32
BF16 = mybir.dt.bfloat16
F8 = mybir.dt.float8e4
AF = mybir.ActivationFunctionType
DR = mybir.MatmulPerfMode.DoubleRow

_CACHE = {}


def _build():
    nc = bacc.Bacc("TRN2", target_bir_lowering=False, debug=False, num_devices=8)

    # wpack/bqk are pre-transposed on host to land as large contiguous
    # per-partition DMA descriptors (strided row-gather descriptors measured
    # ~60 GB/s vs ~350 GB/s contiguous).
    x_d = nc.dram_tensor("x", [C, L], F32, kind="ExternalInput").ap()
    w_d = nc.dram_tensor("wpack", [P, 3 * CO * C], F32, kind="ExternalInput").ap()
    b_d = nc.dram_tensor("bqk", [P, 2 * CO], F32, kind="ExternalInput").ap()
    bv_d = nc.dram_tensor("bvt", [C], F32, kind="ExternalInput").ap()
    y_d = nc.dram_tensor("y", [C, L], F32, kind="ExternalOutput").ap()

    x3 = x_d.rearrange("(co ci) l -> ci co l", ci=P)      # [128, 2, 4096]
    y3 = y_d.rearrange("(co ci) l -> ci co l", ci=P)
    w4 = w_d.rearrange("ci (w cc o) -> ci w cc o", w=3, cc=CO)  # [128,3,2,256]

    with tile.TileContext(nc) as tc:
        with tc.tile_pool(name="consts", bufs=1) as consts, \
             tc.tile_pool(name="big", bufs=1) as big, \
             tc.tile_pool(name="e", bufs=5) as e_pool, \
             tc.tile_pool(name="tmp", bufs=4) as tmp_pool, \
             tc.tile_pool(name="row", bufs=4) as row_pool, \
             tc.tile_pool(name="outp", bufs=3) as out_pool, \
             tc.tile_pool(name="psa", bufs=2, space="PSUM") as ps_a, \
             tc.tile_pool(name="psyh", bufs=2, space="PSUM") as ps_yh, \
             tc.tile_pool(name="psz", bufs=1, space="PSUM") as ps_z, \
             tc.tile_pool(name="pssm", bufs=1, space="PSUM") as ps_small:

            # ---- tiny constants (no DMA dependency) ----
            scratch = consts.tile([P, LB], BF16)          # warm-up operand
            nc.vector.memset(scratch, 0.25)
            ones3_f = consts.tile([P, CO, 16], F32)       # DoubleRow Z lhsT
            nc.vector.memset(ones3_f, 1.0)
            ones3 = consts.tile([P, CO, 16], F8)
            nc.vector.tensor_copy(ones3, ones3_f)
            ones_row_f = consts.tile([1, P], F32)         # lhsT for broadcasts
            nc.vector.memset(ones_row_f, 1.0)
            ones_row = consts.tile([1, P], BF16)
            nc.vector.tensor_copy(ones_row, ones_row_f)
            neg2 = consts.tile([P, 1], F32)               # exp shift bias
            nc.vector.memset(neg2, -2.0)

            # ---- HAM warm-up: dummy matmuls keep the PE busy during the
            # initial DMA wait so the clock-gate opens before real work ----
            warm = ps_a.tile([P, CO, LB], F32, tag="sp", name="warm")
            for i in range(20):
                nc.tensor.matmul(warm[:, i % 2, 0:C], scratch[:, 0:P],
                                 scratch[:, 0:C], start=True, stop=True)

            # ---- input DMAs: x serial on sync (sequential completion at
            # full HBM rate); packed weights + biases on scalar ----
            x_f32 = big.tile([P, CO, L], F32)
            x_f8 = big.tile([P, CO, L], F8)               # q-proj rhs planes
            w_f = consts.tile([P, 3, CO, C], F32)
            bqk_sb = consts.tile([P, 2 * CO], F32)
            bv_f = consts.tile([1, C], F32)
            nc.sync.dma_start(out=w_f, in_=w4)
            nc.sync.dma_start(out=bqk_sb, in_=b_d)
            nc.sync.dma_start(out=bv_f, in_=bv_d[None, :])
            for j in range(0, XC):
                sl = slice(j * XL, (j + 1) * XL)
                nc.sync.dma_start(out=x_f32[:, :, sl], in_=x3[:, :, sl])

            w_f8 = consts.tile([P, 3, CO, C], F8)
            nc.vector.tensor_copy(w_f8, w_f)
            wq8, wk8, wv8 = w_f8[:, 0], w_f8[:, 1], w_f8[:, 2]
            bv_r = consts.tile([1, C], BF16)
            nc.vector.tensor_copy(bv_r, bv_f)

            # ---- persistent activations ----
            xh_f8 = big.tile([P, CO, M], F8)              # Haar high band
            q_f8 = big.tile([P, CO, L], F8)               # [c, l] planes
            k_f8 = big.tile([P, CO, M], F8)               # [c, m] planes
            vt_f8 = big.tile([P, MJ, C], F8)              # [m, o] chunks

            EXP_SCALE = float(1.0 / np.sqrt(np.float32(C)))

            # ---------------- attention task definitions ----------------
            # Flat stream over (lt, g): producer = scores + exp for group g
            # (256 keys); consumer (LAGP groups later) = Z and v@E matmuls;
            # epilogue(lt-1) emitted at (lt, g==2) so the normalization
            # chain overlaps the next tile's score matmuls.
            LAGP = 3
            pend = {}
            yhps = {}
            zts = {}

            def producer(idx):
                lt, g = divmod(idx, G)
                sl = slice(lt * LB, (lt + 1) * LB)
                sp = ps_a.tile([P, CO, LB], F32, tag="sp", name=f"sp{lt}_{g}")
                for h in range(CO):
                    mj = 2 * g + h
                    nc.tensor.matmul(
                        sp[:, h, :], k_f8[:, :, mj * P:(mj + 1) * P],
                        q_f8[:, :, sl], start=True, stop=True, perf_mode=DR)
                e = e_pool.tile([P, CO, LB], F8, tag="e", name=f"e{lt}_{g}")
                # exp((S - 2*sqrt(C)) / sqrt(C)): -2 shift keeps E in fp8
                # range; cancels exactly in the softmax normalization.
                nc.scalar.activation(e, sp, AF.Exp, scale=EXP_SCALE, bias=neg2)
                pend[idx] = e

            def consumer(idx):
                lt, g = divmod(idx, G)
                e = pend.pop(idx)
                if g == 0:
                    zts[lt] = ps_z.tile([P, LB], F32, tag="z", name=f"z{lt}")
                    yhps[lt] = [ps_yh.tile([P, LB], F32, tag="yh",
                                           name=f"yh{lt}_{i}") for i in range(CO)]
                nc.tensor.matmul(zts[lt][0:1, :], ones3[:, :, 0:1], e,
                                 start=(g == 0), stop=(g == G - 1), perf_mode=DR)
                for oc in range(CO):
                    nc.tensor.matmul(
                        yhps[lt][oc], vt_f8[:, 2 * g:2 * g + 2, oc * P:(oc + 1) * P],
                        e, start=(g == 0), stop=(g == G - 1), perf_mode=DR)

            rlos = {}

            def epilogue_a(lt):
                # reciprocal chain; must follow consumer(lt, G-1) emission
                rz = row_pool.tile([1, LB], F32, tag="rz")
                nc.vector.reciprocal_approx_fast(out=rz, in_=zts[lt][0:1, :])
                r_hi = row_pool.tile([1, LB], BF16, tag="rhi")
                nc.vector.tensor_copy(r_hi, rz)
                r_lo = row_pool.tile([1, LB], BF16, tag="rlo")
                nc.vector.tensor_sub(r_lo, rz, r_hi)
                rlos[lt] = (r_hi, r_lo)

            def epilogue_b(lt):
                sl = slice(lt * LB, (lt + 1) * LB)
                r_hi, r_lo = rlos.pop(lt)
                bp = ps_small.tile([P, LB], F32, tag="sm", name=f"bp{lt}")
                nc.tensor.matmul(bp, ones_row, r_hi, start=True, stop=False)
                nc.tensor.matmul(bp, ones_row, r_lo, start=False, stop=True)
                b_sb = tmp_pool.tile([P, LB], F32, tag="bsb")
                nc.vector.tensor_copy(b_sb, bp)
                o_sb = out_pool.tile([P, CO, LB], F32, tag="o")
                if lt < NB - 1:
                    for oc in range(CO):
                        t_sb = tmp_pool.tile([P, LB], F32, tag="t")
                        nc.vector.tensor_mul(t_sb, yhps[lt][oc], b_sb)
                        nc.vector.tensor_add(o_sb[:, oc, :], t_sb,
                                             x_f32[:, oc, sl])
                        (nc.sync if oc == 0 else nc.scalar).dma_start(
                            out=y3[:, oc, sl], in_=o_sb[:, oc, :])
                else:
                    # last tile: quarter-granularity so the first output
                    # bytes leave while the rest is still being computed
                    # (the kernel-exit barrier waits on the final DMA)
                    H = LB // 2
                    for oc in range(CO):
                        for h in range(2):
                            hs = slice(h * H, (h + 1) * H)
                            gs = slice(lt * LB + h * H, lt * LB + (h + 1) * H)
                            t_sb = tmp_pool.tile([P, LB], F32, tag="t")
                            nc.vector.tensor_mul(t_sb[:, hs],
                                                 yhps[lt][oc][:, hs],
                                                 b_sb[:, hs])
                            nc.vector.tensor_add(o_sb[:, oc, hs], t_sb[:, hs],
                                                 x_f32[:, oc, gs])
                            (nc.sync if h == 0 else nc.scalar).dma_start(
                                out=y3[:, oc, gs], in_=o_sb[:, oc, hs])

            # ---- head: per x-chunk interleaved projections + l-tile 0 ----
            # x chunk J covers l [1024J, 1024J+1024) -> xh/key range
            # [512J, 512J+512) = k-bank J, vt chunks 4J..4J+3, and unlocks
            # attention groups 2J, 2J+1 of l-tile 0.
            for J in range(XC):
                xsl = slice(J * XL, (J + 1) * XL)
                # Haar band first: it gates the k/vt projection chain
                pair = x_f32[:, :, xsl].rearrange("p c (m two) -> p c m two",
                                                  two=2)
                msl = slice(J * (XL // 2), (J + 1) * (XL // 2))
                nc.vector.tensor_sub(xh_f8[:, :, msl], pair[:, :, :, 0],
                                     pair[:, :, :, 1])
                nc.vector.tensor_copy(x_f8[:, :, xsl], x_f32[:, :, xsl])
                # q projection for both 512-l banks of this chunk
                for s in range(2):
                    sl = slice(J * XL + s * LB, J * XL + (s + 1) * LB)
                    for oc in range(CO):
                        qp = ps_a.tile([P, CO, LB], F32, tag="sp",
                                       name=f"qp{J}_{s}_{oc}")
                        nc.tensor.matmul(qp[:, 0, :],
                                         wq8[:, :, oc * P:(oc + 1) * P],
                                         x_f8[:, :, sl], start=True, stop=True,
                                         perf_mode=DR)
                        nc.scalar.activation(q_f8[:, oc, sl], qp[:, 0, :],
                                             AF.Identity,
                                             bias=bqk_sb[:, oc:oc + 1])
                # k bank J
                for oc in range(CO):
                    kp = ps_a.tile([P, CO, LB], F32, tag="sp",
                                   name=f"kp{J}_{oc}")
                    nc.tensor.matmul(kp[:, 0, :],
                                     wk8[:, :, oc * P:(oc + 1) * P],
                                     xh_f8[:, :, msl], start=True, stop=True,
                                     perf_mode=DR)
                    nc.vector.tensor_scalar_add(k_f8[:, oc, msl], kp[:, 0, :],
                                                bqk_sb[:, CO + oc:CO + oc + 1])
                # vt chunks
                for mj in range(4 * J, 4 * J + 4):
                    ms = slice(mj * P, (mj + 1) * P)
                    vp = ps_small.tile([P, LB], F32, tag="sm", name=f"vp{mj}")
                    nc.tensor.matmul(vp[:, 0:C], xh_f8[:, :, ms], wv8,
                                     start=True, stop=False, perf_mode=DR)
                    nc.tensor.matmul(vp[:, 0:C], ones_row, bv_r,
                                     start=False, stop=True,
                                     skip_group_check=True)
                    nc.vector.tensor_copy(vt_f8[:, mj, :], vp[:, 0:C])
                # attention l-tile 0: groups unlocked by this chunk
                for g in (2 * J, 2 * J + 1):
                    producer(g)
                    if g - LAGP >= 0:
                        consumer(g - LAGP)

            # ---- attention main loop: l-tiles 1..7 (flat pipeline) ----
            for idx in range(G, NB * G):
                lt, g = divmod(idx, G)
                producer(idx)
                consumer(idx - LAGP)
                # consumer(lt-1, G-1) was just emitted at g == LAGP-1: the
                # recip chain may now read Z; the broadcast/multiply half
                # follows one group later so the PE never waits on it.
                if g == LAGP - 1:
                    epilogue_a(lt - 1)
                elif g == LAGP:
                    epilogue_b(lt - 1)
            consumer(NB * G - 3)
            consumer(NB * G - 2)
            consumer(NB * G - 1)
            epilogue_a(NB - 1)
            epilogue_b(NB - 1)

    nc.compile()
    return nc


def _get_nc():
    if "nc" not in _CACHE:
        _CACHE["nc"] = _build()
    return _CACHE["nc"]


def kernel(x, Wq, bq, Wk, bk, Wv, bv, attn_gate, _run_kwargs=None):
    x = np.asarray(x, dtype=np.float32)
    Wq = np.asarray(Wq, dtype=np.float32)
    Wk = np.asarray(Wk, dtype=np.float32)
    Wv = np.asarray(Wv, dtype=np.float32)
    bq = np.asarray(bq, dtype=np.float32)
    bk = np.asarray(bk, dtype=np.float32)
    bv = np.asarray(bv, dtype=np.float32)
    gate = float(np.tanh(np.asarray(attn_gate, dtype=np.float64))[0])

    # lhsT layouts [c_in, c_out]; fold haar 1/sqrt(2) into k and v,
    # tanh(gate) into v. 1/sqrt(C) is folded into the exp scale on device.
    wqT = np.ascontiguousarray(Wq.T).astype(np.float32)
    wkT = np.ascontiguousarray(Wk.T * np.float32(INV_SQRT2)).astype(np.float32)
    wvT = np.ascontiguousarray(Wv.T * np.float32(INV_SQRT2 * gate)).astype(np.float32)
    # [128, 3*2*256]: per-partition contiguous (fast DMA descriptors)
    wpack = np.ascontiguousarray(
        np.stack([wqT, wkT, wvT]).reshape(3, CO, P, C)
        .transpose(2, 0, 1, 3).reshape(P, 3 * CO * C))
    bqk = np.ascontiguousarray(np.concatenate(
        [bq.reshape(CO, P).T, bk.reshape(CO, P).T], axis=1))

    nc = _get_nc()
    in_maps = [{
        "x": np.ascontiguousarray(x[b]),
        "wpack": wpack, "bqk": bqk,
        "bvt": (bv * np.float32(gate)).astype(np.float32),
    } for b in range(B)]
    res = bass_utils.run_bass_kernel_spmd(
        nc, in_maps, core_ids=list(range(B)), **(_run_kwargs or {}))
    out = np.stack([res.results[b]["y"] for b in range(B)]).astype(np.float32)
    if _run_kwargs:
        kernel.last_results = res
    return out
